# revision 22
# baseline (speedup 1.0000x reference)
"""MoE expert-gating kernel for 8 Trainium2 NeuronCores.

Problem (nn_ExpertGating): router MLP (H->H relu, H->E) + softmax + top-2
gating + weighted combine of per-expert outputs.

Sharding: data-parallel over the B*S=8192 tokens -> 1024 tokens per core.
Each core runs the full router for its tokens and combines its slice of all
8 experts' outputs.  No collectives needed; host concatenates the slices.

Per-core pipeline (T=1024 tokens, H=1024, E=8):
  1. transpose x via PE, split x^T into fp16 hi/lo halves (lo scaled 2^13)
  2. hT = relu(W1.T @ xT + b1) via 3 fp16 matmul passes (hi*hi into a main
     PSUM; hi*lo + lo*hi into a correction PSUM, recombined as
     main + corr/2^13) -> exact to ~2^-22 but at full bf16-rate on the PE
     array instead of fp32's half-rate two-pass mode
  3. logitsT[e, t] = W2.T @ hT + b2 (fp32, W2 stationary -> tiny LDW)
  4. transpose logit chunks back to [t, E], softmax, top-2 via max8 +
     max_index -> top-2 gate values + expert ids
  5. indirect-DMA gather of each token's 2 selected expert rows (8 MB
     instead of 32 MB dense), combine out[t] = g0*row0 + g1*row1

The token range is processed in segments of [4, 3, 1] chunks so early
segments' gather+combine overlap later segments' matmuls, and the serial
tail after the last matmul covers only 128 tokens.  fp32-accurate logits
are required: the min top-2/3 margin on this data is ~5e-6; fp16x3 keeps
logit error below ~1e-6.
"""

import numpy as np

B, S, H, E = 4, 2048, 1024, 8
N_CORES = 8
T = (B * S) // N_CORES  # tokens per core
P = 128  # partitions
TCH = T // P  # token chunks per core (8)
KT = H // P  # contraction tiles (8)
HAL = 512  # psum pad width for stage2/3 tiles
SEGS = [(0, 4), (4, 7), (7, 8)]

_compiled_nc = None


def _build():
    import concourse.bacc as bacc
    import concourse.bass as bass
    import concourse.tile as tile
    from concourse import mybir

    f32 = mybir.dt.float32
    f16 = mybir.dt.float16
    u32 = mybir.dt.uint32
    nc = bacc.Bacc("TRN2", target_bir_lowering=False, debug=False,
                   num_devices=N_CORES)

    xhi = nc.dram_tensor("xhi", [KT, T, P], f16, kind="ExternalInput").ap()
    xlo = nc.dram_tensor("xlo", [KT, T, P], f16, kind="ExternalInput").ap()
    eo = nc.dram_tensor("eo", [E * T, H], f32, kind="ExternalInput").ap()
    w1h = nc.dram_tensor("w1h", [H, H], f16, kind="ExternalInput").ap()
    w1l = nc.dram_tensor("w1l", [H, H], f16, kind="ExternalInput").ap()
    b1 = nc.dram_tensor("b1", [H], f32, kind="ExternalInput").ap()
    w2 = nc.dram_tensor("w2", [H, E], f32, kind="ExternalInput").ap()
    b2 = nc.dram_tensor("b2", [E], f32, kind="ExternalInput").ap()
    identd = nc.dram_tensor("ident", [P, P], f32, kind="ExternalInput").ap()
    iotad = nc.dram_tensor("iota", [P, 1], u32, kind="ExternalInput").ap()
    out = nc.dram_tensor("out", [T, H], f32, kind="ExternalOutput").ap()

    with tile.TileContext(nc) as tc:
        with (
            tc.tile_pool(name="singles", bufs=1) as singles,
            tc.tile_pool(name="eopool", bufs=4) as eopool,
            tc.tile_pool(name="accpool", bufs=3) as accpool,
            tc.tile_pool(name="smalls", bufs=8) as smalls,
            tc.tile_pool(name="ltpool", bufs=2) as ltpool,
            tc.tile_pool(name="hprepool", bufs=2) as hprepool,
            tc.tile_pool(name="psum2", bufs=3, space="PSUM") as psum2,
            tc.tile_pool(name="psum2c", bufs=3, space="PSUM") as psum2c,
            tc.tile_pool(name="psum3", bufs=1, space="PSUM") as psum3,
        ):
            # constants first on the SP ring (tiny); the Pool/SWDGE ring is
            # reserved for the 16 gathers -- extra SWDGE ops would push the
            # last gathers past a sem-lane recycle DRAIN in the tail
            ident = singles.tile([P, P], f32)
            nc.sync.dma_start(out=ident[:], in_=identd)
            iota_u = singles.tile([P, 1], u32)
            nc.sync.dma_start(out=iota_u[:], in_=iotad)

            xThi = singles.tile([P, KT, T], f16)  # fp16 high half of x^T
            xTlo = singles.tile([P, KT, T], f16)  # fp16 low half of x^T, x8192

            def load_xT(half):
                # DMA-xbar transpose (2-byte dtype): [512 tok, 128 h] DRAM ->
                # [128 h, 512 tok] SBUF, per k-tile and hi/lo tensor
                tsl = slice(half * (T // 2), (half + 1) * (T // 2))
                for k in range(KT):
                    nc.sync.dma_start(
                        out=xThi[:, k, tsl],
                        in_=xhi[k, tsl, :], transpose=True)
                    nc.sync.dma_start(
                        out=xTlo[:, k, tsl],
                        in_=xlo[k, tsl, :], transpose=True)

            load_xT(0)
            w1h_sb = singles.tile([P, KT, H], f16)  # fp16 high half of W1
            nc.sync.dma_start(out=w1h_sb[:], in_=w1h.rearrange("(k p) m -> p k m", p=P))
            w1l_sb = singles.tile([P, KT, H], f16)  # fp16 low half of W1, x8192
            nc.sync.dma_start(out=w1l_sb[:], in_=w1l.rearrange("(k p) m -> p k m", p=P))
            w2_sb = singles.tile([P, KT, E], f32)  # w2_sb[p,k,e] = W2[k*128+p, e]
            nc.sync.dma_start(out=w2_sb[:], in_=w2.rearrange("(k p) e -> p k e", p=P))
            b1_sb = singles.tile([P, KT], f32)  # b1_sb[p,m] = b1[m*128+p]
            nc.sync.dma_start(out=b1_sb[:], in_=b1.rearrange("(m p) -> p m", p=P))
            b2_sb = singles.tile([E, 1], f32)  # b2 per partition (expert) for stage3
            nc.sync.dma_start(out=b2_sb[:], in_=b2[:, None])
            load_xT(1)

            hT = singles.tile([P, KT, T], f32)  # hT[p,m,t] = relu(x@W1+b1)[t, m*128+p]

            for c0, c1 in SEGS:
                sl = slice(c0 * P, c1 * P)
                W = (c1 - c0) * P
                # ---- stage 2: hT = relu(W1.T @ xT + b1), fp16 x3 passes:
                # main = xhi*whi ; corr = (xhi*wlo' + xlo'*whi), lo pre-scaled
                # by 2^13 -> hT = relu(main + corr/2^13 + b1)
                for m in range(KT):
                    msl = slice(m * P, (m + 1) * P)
                    ps = psum2.tile([P, W], f32, tag="ps", name="ps",
                                    padded_shape=[P, HAL])
                    for k in range(KT):
                        nc.tensor.matmul(
                            ps[:], lhsT=w1h_sb[:, k, msl], rhs=xThi[:, k, sl],
                            start=(k == 0), stop=(k == KT - 1),
                        )
                    psc = psum2c.tile([P, W], f32, tag="psc", name="psc",
                                      padded_shape=[P, HAL])
                    for k in range(KT):
                        nc.tensor.matmul(
                            psc[:], lhsT=w1l_sb[:, k, msl], rhs=xThi[:, k, sl],
                            start=(k == 0), stop=False,
                        )
                        nc.tensor.matmul(
                            psc[:], lhsT=w1h_sb[:, k, msl], rhs=xTlo[:, k, sl],
                            start=False, stop=(k == KT - 1),
                        )
                    hcorr = hprepool.tile([P, W], f32, tag="hcorr", name="hcorr",
                                          padded_shape=[P, HAL])
                    nc.scalar.mul(out=hcorr[:], in_=psc[:], mul=1.0 / 8192.0)
                    hpre = hprepool.tile([P, W], f32, tag="hpre", name="hpre",
                                         padded_shape=[P, HAL])
                    nc.vector.tensor_tensor(out=hpre[:], in0=ps[:], in1=hcorr[:],
                                            op=mybir.AluOpType.add)
                    nc.scalar.activation(
                        out=hT[:, m, sl], in_=hpre[:],
                        func=mybir.ActivationFunctionType.Relu,
                        bias=b1_sb[:, m:m + 1], scale=1.0,
                    )

                # ---- stage 3: logitsT[e, seg] = W2.T @ hT (+ b2) ----
                ps3 = psum3.tile([E, W], f32, tag="ps3", name="ps3",
                                 padded_shape=[E, HAL])
                for k in range(KT):
                    nc.tensor.matmul(
                        ps3[:], lhsT=w2_sb[:, k, :], rhs=hT[:, k, sl],
                        start=(k == 0), stop=(k == KT - 1),
                    )
                lT = ltpool.tile([E, W], f32, tag="lT", name="lT",
                                 padded_shape=[E, HAL])
                nc.scalar.activation(out=lT[:], in_=ps3[:],
                                     func=mybir.ActivationFunctionType.Identity,
                                     bias=b2_sb[:, 0:1], scale=1.0)

                # ---- stage 4+5 per 128-token chunk: softmax, top-2,
                # indirect gather of the 2 selected expert rows, combine ----
                for tch in range(c0, c1):
                    a = tch - c0
                    pl = psum3.tile([P, E], f32, tag="ps3", name="pl", padded_shape=[P, HAL])
                    nc.tensor.transpose(pl[:], lT[:, a * P:(a + 1) * P],
                                        ident[:E, :E])
                    negmax = smalls.tile([P, 1], f32, tag="negmax", name="negmax")
                    nc.vector.reduce_max(negmax[:], pl[:],
                                         axis=mybir.AxisListType.X, negate=True)
                    exps = smalls.tile([P, E], f32, tag="exps", name="exps")
                    nc.scalar.activation(exps[:], pl[:],
                                         func=mybir.ActivationFunctionType.Exp,
                                         bias=negmax[:], scale=1.0)
                    ssum = smalls.tile([P, 1], f32, tag="ssum", name="ssum")
                    nc.vector.reduce_sum(ssum[:], exps[:],
                                         axis=mybir.AxisListType.X)
                    rs = smalls.tile([P, 1], f32, tag="rs", name="rs")
                    nc.vector.reciprocal(rs[:], ssum[:])
                    # top-2 of exps == top-2 of probs; gate = exp * (1/sum)
                    mx8 = smalls.tile([P, 8], f32, tag="mx8", name="mx8")
                    nc.vector.max(mx8[:], exps[:])
                    idx8 = smalls.tile([P, 8], u32, tag="idx8", name="idx8")
                    nc.vector.max_index(idx8[:], mx8[:], exps[:])
                    # flat eo row = expert*T + (tch*128 + partition)
                    base = smalls.tile([P, 1], u32, tag="base", name="base")
                    nc.vector.tensor_scalar_add(base[:], iota_u[:], tch * P)
                    rows = smalls.tile([P, 2], u32, tag="rows", name="rows")
                    for s in range(2):
                        nc.vector.tensor_scalar(
                            rows[:, s:s + 1], idx8[:, s:s + 1],
                            scalar1=T, scalar2=None, op0=mybir.AluOpType.mult)
                        nc.vector.tensor_tensor(
                            out=rows[:, s:s + 1], in0=rows[:, s:s + 1],
                            in1=base[:], op=mybir.AluOpType.add)
                    eo_g = eopool.tile([P, 2, H], f32, tag="eog", name="eog")
                    for s in range(2):
                        nc.gpsimd.indirect_dma_start(
                            out=eo_g[:, s, :], out_offset=None, in_=eo,
                            in_offset=bass.IndirectOffsetOnAxis(
                                ap=rows[:, s:s + 1], axis=0))
                    g0 = smalls.tile([P, 1], f32, tag="g0", name="g0")
                    nc.vector.tensor_mul(g0[:], mx8[:, 0:1], rs[:])
                    g1 = smalls.tile([P, 1], f32, tag="g1", name="g1")
                    nc.vector.tensor_mul(g1[:], mx8[:, 1:2], rs[:])
                    acc = accpool.tile([P, H], f32, tag="acc", name="acc")
                    nc.scalar.activation(acc[:], eo_g[:, 0, :],
                                         func=mybir.ActivationFunctionType.Copy,
                                         scale=g0[:])
                    nc.vector.scalar_tensor_tensor(
                        out=acc[:], in0=eo_g[:, 1, :], scalar=g1[:], in1=acc[:],
                        op0=mybir.AluOpType.mult, op1=mybir.AluOpType.add)
                    nc.sync.dma_start(out=out[tch * P:(tch + 1) * P, :],
                                      in_=acc[:])

    nc.compile()
    return nc


def _get_nc():
    global _compiled_nc
    if _compiled_nc is None:
        _compiled_nc = _build()
    return _compiled_nc


def _split_w1(W1):
    w1f = np.asarray(W1, dtype=np.float32)
    w1hi = np.ascontiguousarray(w1f.astype(np.float16))
    w1lo = np.ascontiguousarray(
        ((w1f.astype(np.float64) - w1hi.astype(np.float64)) * 8192.0)
        .astype(np.float16))
    return w1hi, w1lo


def make_in_maps(hidden_states, expert_outputs, W1, b1, W2, b2):
    hs = np.ascontiguousarray(np.asarray(hidden_states, dtype=np.float32)).reshape(B * S, H)
    eo = np.ascontiguousarray(np.asarray(expert_outputs, dtype=np.float32)).reshape(E, B * S, H)
    w1hi, w1lo = _split_w1(W1)
    b1v = np.ascontiguousarray(np.asarray(b1, dtype=np.float32))
    w2 = np.ascontiguousarray(np.asarray(W2, dtype=np.float32))
    b2v = np.ascontiguousarray(np.asarray(b2, dtype=np.float32))
    identv = np.eye(P, dtype=np.float32)
    iotav = np.arange(P, dtype=np.uint32).reshape(P, 1)
    xh = hs.astype(np.float16)
    xl = ((hs.astype(np.float64) - xh.astype(np.float64)) * 8192.0).astype(np.float16)
    # k-major tiling so each per-k transpose DMA reads one contiguous block
    xh = xh.reshape(B * S, KT, P).transpose(1, 0, 2)
    xl = xl.reshape(B * S, KT, P).transpose(1, 0, 2)
    in_maps = []
    for c in range(N_CORES):
        sl = slice(c * T, (c + 1) * T)
        in_maps.append({
            "xhi": np.ascontiguousarray(xh[:, sl, :]),
            "xlo": np.ascontiguousarray(xl[:, sl, :]),
            "eo": np.ascontiguousarray(eo[:, sl, :]).reshape(E * T, H),
            "w1h": w1hi, "w1l": w1lo, "b1": b1v, "w2": w2, "b2": b2v,
            "ident": identv, "iota": iotav,
        })
    return in_maps


def kernel(hidden_states, expert_outputs, W1, b1, W2, b2, k=2):
    from concourse.bass_utils import run_bass_kernel_spmd

    in_maps = make_in_maps(hidden_states, expert_outputs, W1, b1, W2, b2)
    nc = _get_nc()
    res = run_bass_kernel_spmd(nc, in_maps, core_ids=list(range(N_CORES)))
    full = np.concatenate([res.results[c]["out"] for c in range(N_CORES)], axis=0)
    return full.reshape(B, S, H)


# revision 23
# speedup vs baseline: 1.1351x; 1.1351x over previous
"""MoE expert-gating kernel for 8 Trainium2 NeuronCores.

Problem (nn_ExpertGating): router MLP (H->H relu, H->E) + softmax + top-2
gating + weighted combine of per-expert outputs.

Sharding: data-parallel over the B*S=8192 tokens -> 1024 tokens per core.
Each core runs the full router for its tokens and combines its slice of all
8 experts' outputs.  No collectives needed; host concatenates the slices.

Per-core pipeline (T=1024 tokens, H=1024, E=8):
  1. transpose x via PE, split x^T into fp16 hi/lo halves (lo scaled 2^13)
  2. hT = relu(W1.T @ xT + b1) via 3 fp16 matmul passes (hi*hi into a main
     PSUM; hi*lo + lo*hi into a correction PSUM, recombined as
     main + corr/2^13) -> exact to ~2^-22 but at full bf16-rate on the PE
     array instead of fp32's half-rate two-pass mode
  3. logitsT[e, t] = W2.T @ hT + b2 (fp32, W2 stationary -> tiny LDW)
  4. transpose logit chunks back to [t, E], softmax, top-2 via max8 +
     max_index -> top-2 gate values + expert ids
  5. indirect-DMA gather of each token's 2 selected expert rows (8 MB
     instead of 32 MB dense), combine out[t] = g0*row0 + g1*row1

The token range is processed in segments of [4, 3, 1] chunks so early
segments' gather+combine overlap later segments' matmuls, and the serial
tail after the last matmul covers only 128 tokens.  fp32-accurate logits
are required: the min top-2/3 margin on this data is ~5e-6; fp16x3 keeps
logit error below ~1e-6.
"""

import numpy as np

B, S, H, E = 4, 2048, 1024, 8
N_CORES = 8
T = (B * S) // N_CORES  # tokens per core
P = 128  # partitions
TCH = T // P  # token chunks per core (8)
KT = H // P  # contraction tiles (8)
HAL = 512  # psum pad width for stage2/3 tiles
SEGS = [(0, 4), (4, 7), (7, 8)]

_compiled_nc = None


def _build():
    import concourse.bacc as bacc
    import concourse.bass as bass
    import concourse.tile as tile
    from concourse import mybir

    f32 = mybir.dt.float32
    f16 = mybir.dt.float16
    u32 = mybir.dt.uint32
    nc = bacc.Bacc("TRN2", target_bir_lowering=False, debug=False,
                   num_devices=N_CORES)

    x = nc.dram_tensor("x", [T, H], f32, kind="ExternalInput").ap()
    eo = nc.dram_tensor("eo", [E * T, H], f32, kind="ExternalInput").ap()
    w1h = nc.dram_tensor("w1h", [H, H], f16, kind="ExternalInput").ap()
    w1l = nc.dram_tensor("w1l", [H, H], f16, kind="ExternalInput").ap()
    b1 = nc.dram_tensor("b1", [H], f32, kind="ExternalInput").ap()
    w2 = nc.dram_tensor("w2", [H, E], f32, kind="ExternalInput").ap()
    b2 = nc.dram_tensor("b2", [E], f32, kind="ExternalInput").ap()
    identd = nc.dram_tensor("ident", [P, P], f32, kind="ExternalInput").ap()
    iotad = nc.dram_tensor("iota", [P, 1], u32, kind="ExternalInput").ap()
    out = nc.dram_tensor("out", [T, H], f32, kind="ExternalOutput").ap()

    with tile.TileContext(nc) as tc:
        with (
            tc.tile_pool(name="singles", bufs=1) as singles,
            tc.tile_pool(name="xpool", bufs=1) as xpool,
            tc.tile_pool(name="eopool", bufs=4) as eopool,
            tc.tile_pool(name="accpool", bufs=3) as accpool,
            tc.tile_pool(name="smalls", bufs=8) as smalls,
            tc.tile_pool(name="ltpool", bufs=2) as ltpool,
            tc.tile_pool(name="tmppool", bufs=3) as tmppool,
            tc.tile_pool(name="hprepool", bufs=2) as hprepool,
            tc.tile_pool(name="psumT", bufs=2, space="PSUM") as psumT,
            tc.tile_pool(name="psum2", bufs=3, space="PSUM") as psum2,
            tc.tile_pool(name="psum2c", bufs=2, space="PSUM") as psum2c,
            tc.tile_pool(name="psum3", bufs=1, space="PSUM") as psum3,
        ):
            # constants first on the SP ring (tiny); the Pool/SWDGE ring is
            # reserved for the 16 gathers -- extra SWDGE ops would push the
            # last gathers past a sem-lane recycle DRAIN in the tail
            ident = singles.tile([P, P], f32)
            nc.sync.dma_start(out=ident[:], in_=identd)
            iota_u = singles.tile([P, 1], u32)
            nc.sync.dma_start(out=iota_u[:], in_=iotad)

            x_half = [None, None]
            x_half[0] = xpool.tile([P, TCH // 2, H], f32, tag="x4", name="x4a")
            for c in range(2):
                nc.sync.dma_start(
                    out=x_half[0][:, 2 * c:2 * c + 2, :],
                    in_=x[2 * c * P:(2 * c + 2) * P, :].rearrange(
                        "(a p) h -> p a h", p=P))
            w1h_sb = singles.tile([P, KT, H], f16)  # fp16 high half of W1
            nc.sync.dma_start(out=w1h_sb[:], in_=w1h.rearrange("(k p) m -> p k m", p=P))
            w1l_sb = singles.tile([P, KT, H], f16)  # fp16 low half of W1, x8192
            nc.sync.dma_start(out=w1l_sb[:], in_=w1l.rearrange("(k p) m -> p k m", p=P))
            w2_sb = singles.tile([P, KT, E], f32)  # w2_sb[p,k,e] = W2[k*128+p, e]
            nc.sync.dma_start(out=w2_sb[:], in_=w2.rearrange("(k p) e -> p k e", p=P))
            b1_sb = singles.tile([P, KT], f32)  # b1_sb[p,m] = b1[m*128+p]
            nc.sync.dma_start(out=b1_sb[:], in_=b1.rearrange("(m p) -> p m", p=P))
            b2_sb = singles.tile([E, 1], f32)  # b2 per partition (expert) for stage3
            nc.sync.dma_start(out=b2_sb[:], in_=b2[:, None])
            x_half[1] = xpool.tile([P, TCH // 2, H], f32, tag="x4", name="x4b")
            for c in range(2):
                nc.sync.dma_start(
                    out=x_half[1][:, 2 * c:2 * c + 2, :],
                    in_=x[T // 2 + 2 * c * P:T // 2 + (2 * c + 2) * P, :].rearrange(
                        "(a p) h -> p a h", p=P))

            xThi = singles.tile([P, KT, T], f16)  # fp16 high half of x^T
            xTlo = singles.tile([P, KT, T], f16)  # fp16 low half of x^T, x8192
            hT = singles.tile([P, KT, T], f32)  # hT[p,m,t] = relu(x@W1+b1)[t, m*128+p]

            def transpose_batch(tch, kk):
                # transpose 4 k-blocks of token chunk tch into one psum tile,
                # then split into fp16 hi + (scaled) lo halves
                x4 = x_half[tch // (TCH // 2)]
                a = tch % (TCH // 2)
                csl = slice(tch * P, (tch + 1) * P)
                pt = psumT.tile([P, 4 * P], f32, tag="pt", name="pt")
                for j in range(4):
                    k = kk * 4 + j
                    nc.tensor.transpose(pt[:, j * P:(j + 1) * P],
                                        x4[:, a, k * P:(k + 1) * P], ident[:])
                ksl = slice(kk * 4, (kk + 1) * 4)
                pt3 = pt[:].rearrange("p (j c) -> p j c", j=4)
                nc.scalar.copy(out=xThi[:, ksl, csl], in_=pt3)
                tmp = tmppool.tile([P, 4, P], f32, tag="tmp", name="tmp")
                nc.vector.tensor_tensor(out=tmp[:], in0=pt3,
                                        in1=xThi[:, ksl, csl],
                                        op=mybir.AluOpType.subtract)
                nc.vector.tensor_scalar_mul(xTlo[:, ksl, csl], tmp[:], 8192.0)

            # seg0's transposes up front; the rest interleave into stage2
            for tch in range(0, 4):
                for kk in range(KT // 4):
                    transpose_batch(tch, kk)
            pending = [(tch, kk) for tch in range(4, TCH) for kk in range(KT // 4)]

            for c0, c1 in SEGS:
                sl = slice(c0 * P, c1 * P)
                W = (c1 - c0) * P
                # ---- stage 2: hT = relu(W1.T @ xT + b1), fp16 x3 passes:
                # main = xhi*whi ; corr = (xhi*wlo' + xlo'*whi), lo pre-scaled
                # by 2^13 -> hT = relu(main + corr/2^13 + b1)
                for m in range(KT):
                    msl = slice(m * P, (m + 1) * P)
                    ps = psum2.tile([P, W], f32, tag="ps", name="ps",
                                    padded_shape=[P, HAL])
                    for k in range(KT):
                        nc.tensor.matmul(
                            ps[:], lhsT=w1h_sb[:, k, msl], rhs=xThi[:, k, sl],
                            start=(k == 0), stop=(k == KT - 1),
                        )
                    psc = psum2c.tile([P, W], f32, tag="psc", name="psc",
                                      padded_shape=[P, HAL])
                    for k in range(KT):
                        nc.tensor.matmul(
                            psc[:], lhsT=w1l_sb[:, k, msl], rhs=xThi[:, k, sl],
                            start=(k == 0), stop=False,
                        )
                        nc.tensor.matmul(
                            psc[:], lhsT=w1h_sb[:, k, msl], rhs=xTlo[:, k, sl],
                            start=False, stop=(k == KT - 1),
                        )
                    if pending:
                        transpose_batch(*pending.pop(0))
                    hcorr = hprepool.tile([P, W], f32, tag="hcorr", name="hcorr",
                                          padded_shape=[P, HAL])
                    nc.scalar.mul(out=hcorr[:], in_=psc[:], mul=1.0 / 8192.0)
                    hpre = hprepool.tile([P, W], f32, tag="hpre", name="hpre",
                                         padded_shape=[P, HAL])
                    nc.vector.tensor_tensor(out=hpre[:], in0=ps[:], in1=hcorr[:],
                                            op=mybir.AluOpType.add)
                    nc.scalar.activation(
                        out=hT[:, m, sl], in_=hpre[:],
                        func=mybir.ActivationFunctionType.Relu,
                        bias=b1_sb[:, m:m + 1], scale=1.0,
                    )

                # ---- stage 3: logitsT[e, seg] = W2.T @ hT (+ b2) ----
                ps3 = psum3.tile([E, W], f32, tag="ps3", name="ps3",
                                 padded_shape=[E, HAL])
                for k in range(KT):
                    nc.tensor.matmul(
                        ps3[:], lhsT=w2_sb[:, k, :], rhs=hT[:, k, sl],
                        start=(k == 0), stop=(k == KT - 1),
                    )
                lT = ltpool.tile([E, W], f32, tag="lT", name="lT",
                                 padded_shape=[E, HAL])
                nc.scalar.activation(out=lT[:], in_=ps3[:],
                                     func=mybir.ActivationFunctionType.Identity,
                                     bias=b2_sb[:, 0:1], scale=1.0)

                # ---- stage 4+5 per 128-token chunk: softmax, top-2,
                # indirect gather of the 2 selected expert rows, combine ----
                for tch in range(c0, c1):
                    a = tch - c0
                    pl = psum3.tile([P, E], f32, tag="ps3", name="pl", padded_shape=[P, HAL])
                    nc.tensor.transpose(pl[:], lT[:, a * P:(a + 1) * P],
                                        ident[:E, :E])
                    negmax = smalls.tile([P, 1], f32, tag="negmax", name="negmax")
                    nc.vector.reduce_max(negmax[:], pl[:],
                                         axis=mybir.AxisListType.X, negate=True)
                    exps = smalls.tile([P, E], f32, tag="exps", name="exps")
                    nc.scalar.activation(exps[:], pl[:],
                                         func=mybir.ActivationFunctionType.Exp,
                                         bias=negmax[:], scale=1.0)
                    ssum = smalls.tile([P, 1], f32, tag="ssum", name="ssum")
                    nc.vector.reduce_sum(ssum[:], exps[:],
                                         axis=mybir.AxisListType.X)
                    rs = smalls.tile([P, 1], f32, tag="rs", name="rs")
                    nc.vector.reciprocal(rs[:], ssum[:])
                    # top-2 of exps == top-2 of probs; gate = exp * (1/sum)
                    mx8 = smalls.tile([P, 8], f32, tag="mx8", name="mx8")
                    nc.vector.max(mx8[:], exps[:])
                    idx8 = smalls.tile([P, 8], u32, tag="idx8", name="idx8")
                    nc.vector.max_index(idx8[:], mx8[:], exps[:])
                    # flat eo row = expert*T + (tch*128 + partition)
                    base = smalls.tile([P, 1], u32, tag="base", name="base")
                    nc.vector.tensor_scalar_add(base[:], iota_u[:], tch * P)
                    rows = smalls.tile([P, 2], u32, tag="rows", name="rows")
                    for s in range(2):
                        nc.vector.tensor_scalar(
                            rows[:, s:s + 1], idx8[:, s:s + 1],
                            scalar1=T, scalar2=None, op0=mybir.AluOpType.mult)
                        nc.vector.tensor_tensor(
                            out=rows[:, s:s + 1], in0=rows[:, s:s + 1],
                            in1=base[:], op=mybir.AluOpType.add)
                    eo_g = eopool.tile([P, 2, H], f32, tag="eog", name="eog")
                    for s in range(2):
                        nc.gpsimd.indirect_dma_start(
                            out=eo_g[:, s, :], out_offset=None, in_=eo,
                            in_offset=bass.IndirectOffsetOnAxis(
                                ap=rows[:, s:s + 1], axis=0))
                    g0 = smalls.tile([P, 1], f32, tag="g0", name="g0")
                    nc.vector.tensor_mul(g0[:], mx8[:, 0:1], rs[:])
                    g1 = smalls.tile([P, 1], f32, tag="g1", name="g1")
                    nc.vector.tensor_mul(g1[:], mx8[:, 1:2], rs[:])
                    acc = accpool.tile([P, H], f32, tag="acc", name="acc")
                    nc.scalar.activation(acc[:], eo_g[:, 0, :],
                                         func=mybir.ActivationFunctionType.Copy,
                                         scale=g0[:])
                    nc.vector.scalar_tensor_tensor(
                        out=acc[:], in0=eo_g[:, 1, :], scalar=g1[:], in1=acc[:],
                        op0=mybir.AluOpType.mult, op1=mybir.AluOpType.add)
                    nc.sync.dma_start(out=out[tch * P:(tch + 1) * P, :],
                                      in_=acc[:])

    nc.compile()
    return nc


def _get_nc():
    global _compiled_nc
    if _compiled_nc is None:
        _compiled_nc = _build()
    return _compiled_nc


def _split_w1(W1):
    w1f = np.asarray(W1, dtype=np.float32)
    w1hi = np.ascontiguousarray(w1f.astype(np.float16))
    w1lo = np.ascontiguousarray(
        ((w1f.astype(np.float64) - w1hi.astype(np.float64)) * 8192.0)
        .astype(np.float16))
    return w1hi, w1lo


def make_in_maps(hidden_states, expert_outputs, W1, b1, W2, b2):
    hs = np.ascontiguousarray(np.asarray(hidden_states, dtype=np.float32)).reshape(B * S, H)
    eo = np.ascontiguousarray(np.asarray(expert_outputs, dtype=np.float32)).reshape(E, B * S, H)
    w1hi, w1lo = _split_w1(W1)
    b1v = np.ascontiguousarray(np.asarray(b1, dtype=np.float32))
    w2 = np.ascontiguousarray(np.asarray(W2, dtype=np.float32))
    b2v = np.ascontiguousarray(np.asarray(b2, dtype=np.float32))
    identv = np.eye(P, dtype=np.float32)
    iotav = np.arange(P, dtype=np.uint32).reshape(P, 1)
    in_maps = []
    for c in range(N_CORES):
        sl = slice(c * T, (c + 1) * T)
        in_maps.append({
            "x": np.ascontiguousarray(hs[sl]),
            "eo": np.ascontiguousarray(eo[:, sl, :]).reshape(E * T, H),
            "w1h": w1hi, "w1l": w1lo, "b1": b1v, "w2": w2, "b2": b2v,
            "ident": identv, "iota": iotav,
        })
    return in_maps


def kernel(hidden_states, expert_outputs, W1, b1, W2, b2, k=2):
    from concourse.bass_utils import run_bass_kernel_spmd

    in_maps = make_in_maps(hidden_states, expert_outputs, W1, b1, W2, b2)
    nc = _get_nc()
    res = run_bass_kernel_spmd(nc, in_maps, core_ids=list(range(N_CORES)))
    full = np.concatenate([res.results[c]["out"] for c in range(N_CORES)], axis=0)
    return full.reshape(B, S, H)
